# revision 11
# baseline (speedup 1.0000x reference)
"""Trainium2 Bass kernel for CausalDownsamplingLRU.

Algorithm (per core = one batch element; 8 cores, data-parallel over batch):
  With lam = r*e^{i theta} (per state n), h_t = lam*h_{t-1} + Bu_t, and only
  y[:, -DS:] needed:

  1. Input GEMMs (fp16, PE): Bu^T[n,t] = (gamma*B)^T.T @ x^T  (re & im planes)
  2. FIRST half: only h_{1023} (the carry into the output window) is needed,
     truncated to the last WCON=256 steps (error ~ r^256, negligible in norm):
        w[i,n]  = sum_s x_s[i] V[s,n],  V = lam^{255-s}     (PE, flipped GEMM)
        a[n]    = sum_i (gamma*B)[n,i] (.) w[i,n]           (PE diag-block MMs
                  into PSUM [n_p, n_f]; diagonal extracted by a masked
                  STT-reduce against an identity tile)
  3. SECOND half: phase twist e_j = e^{-i j theta} (.) Bu_{1024+j} decouples
     the complex recurrence into two REAL per-partition scans
        s_j = r*s_{j-1} + e_j   (tensor_tensor_scan, fp32 state),
     with initial s_{-1} = e^{i theta} * h_1023.
  4. Untwist h = e^{+i j theta} (.) s, then output GEMMs:
        y^T = C_re^T.T @ h_re + (-C_im^T).T @ h_im + D^T.T @ x^T

Perf notes (v4):
  - Twists/untwists run full-width [P, 1024]; all elementwise work on DVE
    (GPSIMD compute and DMA-accum combines both measured slower end-to-end).
  - Loads ride ONLY the two HWDGE queues (sync/scalar) in need-time order;
    the gpsimd SWDGE queue measured 3-5x slower and only takes stores.
  - The carry path (wvv = xw|vre|vim) is front-loaded so the first scan is
    not DMA-gated; the W-GEMM runs before the first input GEMM on PE.
  - btiN (host-negated gamma*B_im) removes the negated-wim evac copies from
    the Act engine's critical early window.
"""
import numpy as np

import concourse.bass as bass
import concourse.bacc as bacc
import concourse.mybir as mybir
from concourse.tile import TileContext
from concourse.bass_utils import run_bass_kernel_spmd

BATCH, T, IN, OUT, N = 8, 2048, 512, 512, 512
DS = 1024
P = 128
NB = N // P    # 4 state blocks
IBN = IN // P  # 4 input blocks
OBN = OUT // P # 4 output blocks
HF = 1024      # window length (= DS)
HH = 512       # half length (PSUM bank limit for f32 out)
WCON = 256     # carry W-GEMM contraction (last WCON steps of first half)
WB = WCON // P

f32 = mybir.dt.float32
f16 = mybir.dt.float16
AOP = mybir.AluOpType

_CACHE = {}


def _build_nc():
    if "nc" in _CACHE:
        return _CACHE["nc"]
    nc = bacc.Bacc()
    xT = nc.dram_tensor("xT", [IN, HF], f16, kind="ExternalInput")    # 2nd half, [i, t]
    # wvv rows s: [ xw[s, 0:IN] | vre[s, 0:N] ];  wvb: [ vim[s, 0:N] ]
    wvv = nc.dram_tensor("wvv", [WCON, IN + N], f16, kind="ExternalInput")
    wvb = nc.dram_tensor("wvb", [WCON, N], f16, kind="ExternalInput")
    # nb-blocked weights: [p, nb*IN] with cols (nb, ib, n-within-block)
    bt01 = nc.dram_tensor("bt01", [P, 2 * IN], f16, kind="ExternalInput")   # btrP0|btiP0
    bt123 = nc.dram_tensor("bt123", [P, 6 * IN], f16, kind="ExternalInput") # btrP1-3|btiP1-3
    btiN = nc.dram_tensor("btiN", [P, 4 * IN], f16, kind="ExternalInput")   # -btiP all nb
    # per-nb trig: rows 0-127 cos, 128-255 sin (so one consolidated DMA each)
    trig = [nc.dram_tensor(f"trig{nb}", [2 * P, HF], f16, kind="ExternalInput")
            for nb in range(NB)]
    eye = nc.dram_tensor("eye", [P, P], f16, kind="ExternalInput")
    # rr columns: 0=r, 1=cos(theta), 2=-sin(theta), 3=sin(theta)
    rr = nc.dram_tensor("rr", [N, 4], f32, kind="ExternalInput")
    ctr = nc.dram_tensor("ctr", [N, OUT], f16, kind="ExternalInput")
    ctin = nc.dram_tensor("ctin", [N, OUT], f16, kind="ExternalInput")
    dtw = nc.dram_tensor("dtw", [IN, OUT], f16, kind="ExternalInput")
    yT = nc.dram_tensor("yT", [OUT, DS], f16, kind="ExternalOutput")

    with TileContext(nc) as tc:
        with (
            tc.tile_pool(name="const", bufs=1) as cp,
            tc.tile_pool(name="work", bufs=1) as wkp,
            tc.tile_pool(name="ps", bufs=2, space="PSUM") as bp,
            tc.tile_pool(name="wps", bufs=2, space="PSUM") as wp,
            tc.tile_pool(name="yps", bufs=4, space="PSUM") as yp,
        ):
            def load_consolidated(dram, eng, tagp, cols=None, col0=0):
                """One rearranged DMA for a [R*P, C] dram table -> row tiles."""
                rows = dram.shape[0]
                nb_ = rows // P
                cols = cols if cols is not None else dram.shape[1]
                big = cp.tile([P, nb_ * cols], dram.dtype, tag=tagp, name=tagp)
                eng.dma_start(
                    big[:].rearrange("p (b c) -> p b c", b=nb_),
                    dram[:, col0:col0 + cols].rearrange("(b p) c -> p b c", p=P))
                return [big[:, i * cols:(i + 1) * cols] for i in range(nb_)]

            trig_t = [None] * NB

            def load_trig(nb, eng):
                big = cp.tile([P, 2 * HF], f16, tag=f"trig{nb}", name=f"trig{nb}")
                eng.dma_start(
                    big[:].rearrange("p (b c) -> p b c", b=2),
                    trig[nb][:, :].rearrange("(b p) c -> p b c", p=P))
                trig_t[nb] = big

            # ---------------- DMA map (need-time order per HWDGE queue) ---
            # scalar (Act HWDGE): carry path + first-block weights first; the
            # queue drains before Act's evac work begins.
            wvv_t = load_consolidated(wvv, nc.scalar, "wvv")  # WB blocks of 1024
            bt01_t = cp.tile([P, 2 * IN], f16, tag="bt01", name="bt01")
            nc.scalar.dma_start(bt01_t[:], bt01[:, :])
            load_trig(0, nc.scalar)
            btiN_t = cp.tile([P, 4 * IN], f16, tag="btiN", name="btiN")
            nc.scalar.dma_start(btiN_t[:], btiN[:, :])
            load_trig(2, nc.scalar)
            ctin_t = load_consolidated(ctin, nc.scalar, "ctin")

            # sync (SP HWDGE): small consts, x halves, remaining weights
            wvb_t = load_consolidated(wvb, nc.sync, "wvb")
            eye_t = cp.tile([P, P], f16, tag="eye", name="eye")
            nc.sync.dma_start(eye_t[:], eye[:, :])
            rr_t = load_consolidated(rr, nc.sync, "rr")     # [P, 4] per nb
            xth0s = load_consolidated(xT, nc.sync, "xth0", cols=HH, col0=0)
            xth1s = load_consolidated(xT, nc.sync, "xth1", cols=HH, col0=HH)
            bt123_t = cp.tile([P, 6 * IN], f16, tag="bt123", name="bt123")
            nc.sync.dma_start(bt123_t[:], bt123[:, :])
            load_trig(1, nc.sync)
            ctr_t = load_consolidated(ctr, nc.sync, "ctr")
            load_trig(3, nc.sync)
            dtw_t = load_consolidated(dtw, nc.sync, "dtw")

            # weight block views
            btrP_t = [bt01_t[:, 0:IN]] + [bt123_t[:, i * IN:(i + 1) * IN]
                                          for i in range(3)]
            btiP_t = [bt01_t[:, IN:2 * IN]] + [bt123_t[:, (3 + i) * IN:(4 + i) * IN]
                                               for i in range(3)]
            btiN_t_ = [btiN_t[:, i * IN:(i + 1) * IN] for i in range(NB)]
            xw_t = [wvv_t[sb][:, 0:IN] for sb in range(WB)]
            vre_t = [wvv_t[sb][:, IN:IN + N] for sb in range(WB)]
            vim_t = wvb_t

            def xv(ib, h):
                """x^T [P, HH] view for input block ib, half h."""
                return (xth0s if h == 0 else xth1s)[ib]

            def cosv(nb):
                return trig_t[nb][:, 0:HF]

            def sinv(nb):
                return trig_t[nb][:, HF:2 * HF]

            # ---------------- PE building blocks ----------------
            bus = {}

            def in_plane(nb, pi, evac_vec=False):
                wt = (btrP_t, btiP_t)[pi][nb]
                bu = wkp.tile([P, HF], f16, tag=f"bu{pi}", bufs=2,
                              name=f"bu{pi}_{nb}")
                for h in range(2):
                    ps = bp.tile([P, HH], f32, tag="ps", name=f"bups{h}")
                    for ib in range(IBN):
                        nc.tensor.matmul(
                            ps[:], wt[:, ib * P:(ib + 1) * P], xv(ib, h),
                            start=(ib == 0), stop=(ib == IBN - 1))
                    if evac_vec:
                        nc.vector.tensor_scalar_mul(
                            bu[:, h * HH:(h + 1) * HH], ps[:], 1.0)
                    else:
                        nc.scalar.copy(bu[:, h * HH:(h + 1) * HH], ps[:])
                bus[(nb, pi)] = bu

            # W-GEMM (flipped): w[i-block, n] = sum_s xw[s, i] * V[s, n]
            wts = {}

            def w_mm(pi):
                vt = (vre_t, vim_t)[pi]
                for ib in range(IBN):
                    ps = wp.tile([P, IN], f32, tag="wps", name="wps")
                    for sb in range(WB):
                        nc.tensor.matmul(
                            ps[:], xw_t[sb][:, ib * P:(ib + 1) * P], vt[sb][:],
                            start=(sb == 0), stop=(sb == WB - 1))
                    w = wkp.tile([P, IN], f16, tag=f"w{pi}{ib}", name=f"w{pi}{ib}")
                    nc.scalar.copy(w[:], ps[:])
                    wts[(pi, ib)] = w

            inits = {}
            diag_aps = {}

            def diag_mm(nb):
                """a_re/a_im via diag-block MMs + masked STT reduce + rotate."""
                nsl = slice(nb * P, (nb + 1) * P)
                bsl = lambda t, ib: t[:, ib * P:(ib + 1) * P]
                aps = {}
                for name in ("are", "aim"):
                    aps[name] = yp.tile([P, HH], f32, tag="yps", name=name)
                # a_re = sum gbr.wre - gbi.wim  (minus via host-negated btiN)
                # a_im = sum gbr.wim + gbi.wre
                for ib in range(IBN):
                    nc.tensor.matmul(aps["are"][:, 0:P], bsl(btrP_t[nb], ib),
                                     wts[(0, ib)][:, nsl],
                                     start=(ib == 0), stop=False,
                                     skip_group_check=True)
                    nc.tensor.matmul(aps["aim"][:, 0:P], bsl(btrP_t[nb], ib),
                                     wts[(1, ib)][:, nsl],
                                     start=(ib == 0), stop=False,
                                     skip_group_check=True)
                for ib in range(IBN):
                    nc.tensor.matmul(aps["are"][:, 0:P], bsl(btiN_t_[nb], ib),
                                     wts[(1, ib)][:, nsl],
                                     start=False, stop=(ib == IBN - 1),
                                     skip_group_check=True)
                    nc.tensor.matmul(aps["aim"][:, 0:P], bsl(btiP_t[nb], ib),
                                     wts[(0, ib)][:, nsl],
                                     start=False, stop=(ib == IBN - 1),
                                     skip_group_check=True)
                diag_aps[nb] = aps

            def diag_fin(nb):
                aps = diag_aps[nb]
                a_re = cp.tile([P, 1], f32, tag=f"are{nb}", name=f"are{nb}")
                a_im = cp.tile([P, 1], f32, tag=f"aim{nb}", name=f"aim{nb}")
                dre = wkp.tile([P, P], f16, tag="dre", name="dre")
                dim = wkp.tile([P, P], f16, tag="dim", name="dim")
                nc.vector.scalar_tensor_tensor(
                    dre[:], aps["are"][:, 0:P], 1.0, eye_t[:], AOP.bypass,
                    AOP.mult, accum_out=a_re[:])
                nc.vector.scalar_tensor_tensor(
                    dim[:], aps["aim"][:, 0:P], 1.0, eye_t[:], AOP.bypass,
                    AOP.mult, accum_out=a_im[:])
                # init = e^{i theta} * a
                rt = rr_t[nb]
                i_re = cp.tile([P, 1], f32, tag=f"ire{nb}", name=f"ire{nb}")
                i_im = cp.tile([P, 1], f32, tag=f"iim{nb}", name=f"iim{nb}")
                u_re = cp.tile([P, 1], f32, tag=f"ure{nb}", name=f"ure{nb}")
                u_im = cp.tile([P, 1], f32, tag=f"uim{nb}", name=f"uim{nb}")
                nc.scalar.mul(u_re[:], a_re[:], rt[:, 1:2])
                nc.vector.scalar_tensor_tensor(
                    i_re[:], a_im[:], rt[:, 2:3], u_re[:], AOP.mult, AOP.add)
                nc.scalar.mul(u_im[:], a_im[:], rt[:, 1:2])
                nc.vector.scalar_tensor_tensor(
                    i_im[:], a_re[:], rt[:, 3:4], u_im[:], AOP.mult, AOP.add)
                inits[nb] = (i_re, i_im)

            # ---------------- DVE building blocks (full width) -----------
            es = {}

            def twist(nb, h=None):
                ct, st = cosv(nb), sinv(nb)
                br, bi = bus[(nb, 0)], bus[(nb, 1)]
                if h is None or h == 0:
                    e_re = wkp.tile([P, HF], f16, tag="er", bufs=4, name=f"er{nb}")
                    e_im = wkp.tile([P, HF], f16, tag="ei", bufs=4, name=f"ei{nb}")
                    es[nb] = (e_re, e_im)
                e_re, e_im = es[nb]
                sl = slice(0, HF) if h is None else slice(h * HH, (h + 1) * HH)
                W = HF if h is None else HH
                sfx = "f" if h is None else "h"
                p1 = wkp.tile([P, W], f16, tag=f"p1{sfx}", bufs=2, name="p1")
                p2 = wkp.tile([P, W], f16, tag=f"p2{sfx}", bufs=2, name="p2")
                nc.vector.tensor_tensor(p1[:], st[:, sl], bi[:, sl], AOP.mult)
                nc.vector.tensor_tensor(p2[:], ct[:, sl], br[:, sl], AOP.mult)
                nc.vector.tensor_tensor(e_re[:, sl], p2[:], p1[:], AOP.add)
                nc.vector.tensor_tensor(p1[:], st[:, sl], br[:, sl], AOP.mult)
                nc.vector.tensor_tensor(p2[:], ct[:, sl], bi[:, sl], AOP.mult)
                nc.vector.tensor_tensor(e_im[:, sl], p2[:], p1[:], AOP.subtract)

            ss = {}

            def scan(nb):
                i_re, i_im = inits[nb]
                e_re, e_im = es[nb]
                s_re = wkp.tile([P, HF], f16, tag="sr", bufs=4, name=f"sr{nb}")
                s_im = wkp.tile([P, HF], f16, tag="si", bufs=4, name=f"si{nb}")
                ss[nb] = (s_re, s_im)
                rbb = rr_t[nb][:, 0:1].broadcast_to((P, HF))
                nc.vector.tensor_tensor_scan(
                    s_re[:], rbb, e_re[:], i_re[:, 0:1], AOP.mult, AOP.add)
                nc.vector.tensor_tensor_scan(
                    s_im[:], rbb, e_im[:], i_im[:, 0:1], AOP.mult, AOP.add)

            hhs = {}

            def untwist(nb, h=None):
                s_re, s_im = ss[nb]
                ct, st = cosv(nb), sinv(nb)
                if h is None or h == 0:
                    hhr = wkp.tile([P, HF], f16, tag="hhr", bufs=4, name=f"hhr{nb}")
                    hhi = wkp.tile([P, HF], f16, tag="hhi", bufs=4, name=f"hhi{nb}")
                    hhs[nb] = (hhr, hhi)
                hhr, hhi = hhs[nb]
                sl = slice(0, HF) if h is None else slice(h * HH, (h + 1) * HH)
                W = HF if h is None else HH
                sfx = "f" if h is None else "h"
                q1 = wkp.tile([P, W], f16, tag=f"q1{sfx}", bufs=2, name="q1")
                q2 = wkp.tile([P, W], f16, tag=f"q2{sfx}", bufs=2, name="q2")
                nc.vector.tensor_tensor(q1[:], ct[:, sl], s_re[:, sl], AOP.mult)
                nc.vector.tensor_tensor(q2[:], st[:, sl], s_im[:, sl], AOP.mult)
                nc.vector.tensor_tensor(hhr[:, sl], q1[:], q2[:], AOP.subtract)
                nc.vector.tensor_tensor(q1[:], ct[:, sl], s_im[:, sl], AOP.mult)
                nc.vector.tensor_tensor(q2[:], st[:, sl], s_re[:, sl], AOP.mult)
                nc.vector.tensor_tensor(hhi[:, sl], q1[:], q2[:], AOP.add)

            # ---------------- output groups ----------------
            groups = {}

            def d_gemms():
                for h in range(2):
                    for ob in range(OBN):
                        gi = h * OBN + ob
                        pool, tag = ((yp, "yps") if gi < 4 else
                                     (bp, "ps") if gi < 6 else (wp, "wps"))
                        groups[(h, ob)] = pool.tile([P, HH], f32, tag=tag, name="yps")
                for ob in range(OBN):
                    osl = slice(ob * P, (ob + 1) * P)
                    for ib in range(IBN):
                        for h in range(2):
                            nc.tensor.matmul(
                                groups[(h, ob)][:], dtw_t[ib][:, osl], xv(ib, h),
                                start=(ib == 0), stop=False)

            store_eng = [nc.sync, nc.gpsimd, nc.scalar]

            def store_group(h, ob):
                osl = slice(ob * P, (ob + 1) * P)
                hsl = slice(h * HH, (h + 1) * HH)
                ysb = wkp.tile([P, HH], f16, tag="ysb", bufs=4, name="ysb")
                gi = h * OBN + ob
                if gi % 2 == 0:
                    nc.scalar.copy(ysb[:], groups[(h, ob)][:])
                else:
                    nc.vector.tensor_scalar_mul(ysb[:], groups[(h, ob)][:], 1.0)
                eng = store_eng[gi % 3]
                eng.dma_start(yT[osl, hsl], ysb[:])

            def c_mms(nb, hs=(0, 1)):
                hhr, hhi = hhs[nb]
                last = nb == NB - 1
                for h in hs:
                    for ob in range(OBN):
                        osl = slice(ob * P, (ob + 1) * P)
                        for wi, (wt, m) in enumerate(
                                ((ctr_t[nb][:, osl], hhr), (ctin_t[nb][:, osl], hhi))):
                            nc.tensor.matmul(
                                groups[(h, ob)][:], wt, m[:, h * HH:(h + 1) * HH],
                                start=False, stop=(last and wi == 1))
                        if last:
                            store_group(h, ob)

            # ---------------- emission ----------------
            w_mm(0)
            in_plane(0, 0)
            in_plane(0, 1, evac_vec=True)
            twist(0, 0)
            w_mm(1)
            twist(0, 1)
            in_plane(1, 0)
            in_plane(1, 1)
            diag_mm(0)
            diag_mm(1)
            diag_fin(0)
            scan(0)
            diag_mm(2)
            diag_mm(3)
            in_plane(2, 0)
            in_plane(2, 1)
            twist(1)
            diag_fin(1)
            untwist(0)
            in_plane(3, 0)
            in_plane(3, 1)
            scan(1)
            diag_fin(2)
            diag_fin(3)
            d_gemms()
            c_mms(0)
            twist(2)
            untwist(1)
            c_mms(1)
            scan(2)
            twist(3)
            untwist(2)
            c_mms(2)
            scan(3)
            untwist(3, 0)
            untwist(3, 1)
            c_mms(3, hs=(0,))
            c_mms(3, hs=(1,))

    nc.compile()
    nc.finalize()
    _CACHE["nc"] = nc
    return nc


def _host_prep(x, nu_log, theta_log, gamma_log, B_re, B_im, C_re, C_im, D):
    f64 = np.float64
    nu = np.asarray(nu_log, f64)
    th = np.asarray(theta_log, f64)
    gl = np.asarray(gamma_log, f64)
    r = np.exp(-np.exp(nu))
    theta = np.exp(th)
    gamma = np.exp(gl)

    gbr = gamma[:, None] * np.asarray(B_re, f64)   # [n, i]
    gbi = gamma[:, None] * np.asarray(B_im, f64)

    def blocked(w_ni):
        """[n, i] -> [p, (nb, ib, nl)] with p = i within block."""
        wt = np.ascontiguousarray(w_ni.T)  # [i, n]
        b = wt.reshape(IBN, P, NB, P)      # (ib, p, nb, nl)
        return np.ascontiguousarray(b.transpose(1, 2, 0, 3).reshape(P, NB * IN)
                                    ).astype(np.float16)

    btrP = blocked(gbr)
    btiP = blocked(gbi)
    shared = {
        "bt01": np.ascontiguousarray(
            np.concatenate([btrP[:, 0:IN], btiP[:, 0:IN]], axis=1)),
        "bt123": np.ascontiguousarray(
            np.concatenate([btrP[:, IN:], btiP[:, IN:]], axis=1)),
        "btiN": np.ascontiguousarray(blocked(-gbi)),
        "ctr": np.ascontiguousarray(np.asarray(C_re, f64).T).astype(np.float16),
        "ctin": np.ascontiguousarray((-np.asarray(C_im, f64)).T).astype(np.float16),
        "dtw": np.ascontiguousarray(np.asarray(D, f64).T).astype(np.float16),
        "eye": np.eye(P, dtype=np.float16),
    }
    j = np.arange(HF, dtype=f64)
    ang = theta[:, None] * j[None, :]
    cosj = np.cos(ang).astype(np.float16)   # [N, HF]
    sinj = np.sin(ang).astype(np.float16)
    for nb in range(NB):
        nsl = slice(nb * P, (nb + 1) * P)
        shared[f"trig{nb}"] = np.ascontiguousarray(
            np.concatenate([cosj[nsl], sinj[nsl]], axis=0))
    # V = lam^{WCON-1-s} over the LAST WCON steps of the first half, [s, n]
    e = (WCON - 1) - np.arange(WCON, dtype=f64)
    mag = np.exp(np.log(r)[:, None] * e[None, :])
    angv = theta[:, None] * e[None, :]
    vre = np.ascontiguousarray((mag * np.cos(angv)).T).astype(np.float16)
    vim = np.ascontiguousarray((mag * np.sin(angv)).T).astype(np.float16)
    shared["rr"] = np.ascontiguousarray(np.stack(
        [r, np.cos(theta), -np.sin(theta), np.sin(theta)],
        axis=1).astype(np.float32))

    x = np.asarray(x, np.float32)
    in_maps = []
    for b in range(BATCH):
        m = dict(shared)
        m["xT"] = np.ascontiguousarray(x[b, HF:].T).astype(np.float16)
        xw = np.ascontiguousarray(x[b, HF - WCON:HF]).astype(np.float16)
        m["wvv"] = np.ascontiguousarray(
            np.concatenate([xw, vre], axis=1))
        m["wvb"] = vim
        in_maps.append(m)
    return in_maps


def _run(in_maps, trace=False):
    nc = _build_nc()
    return run_bass_kernel_spmd(nc, in_maps, core_ids=list(range(BATCH)), trace=trace)


def kernel(**inputs):
    in_maps = _host_prep(**inputs)
    res = _run(in_maps, trace=False)
    y = np.stack([np.ascontiguousarray(res.results[b]["yT"].T) for b in range(BATCH)])
    return y.astype(np.float32)


def kernel_traced(**inputs):
    """Like kernel() but returns (y, exec_time_ns). Used by test.py."""
    in_maps = _host_prep(**inputs)
    res = _run(in_maps, trace=True)
    y = np.stack([np.ascontiguousarray(res.results[b]["yT"].T) for b in range(BATCH)])
    return y.astype(np.float32), res.exec_time_ns


# revision 12
# speedup vs baseline: 1.1847x; 1.1847x over previous
"""Trainium2 Bass kernel for CausalDownsamplingLRU.

Algorithm (per core = one batch element; 8 cores, data-parallel over batch):
  With lam = r*e^{i theta} (per state n), h_t = lam*h_{t-1} + Bu_t, and only
  y[:, -DS:] needed:

  1. Input GEMMs (fp16, PE): Bu^T[n,t] = (gamma*B)^T.T @ x^T  (re & im planes)
  2. FIRST half: only h_{1023} (the carry into the output window) is needed,
     truncated to the last WCON=256 steps (error ~ r^256, negligible in norm):
        w[i,n]  = sum_s x_s[i] V[s,n],  V = lam^{255-s}     (PE, flipped GEMM)
        a[n]    = sum_i (gamma*B)[n,i] (.) w[i,n]           (PE diag-block MMs
                  into PSUM [n_p, n_f]; diagonal extracted by a masked
                  STT-reduce against an identity tile)
  3. SECOND half: phase twist e_j = e^{-i j theta} (.) Bu_{1024+j} decouples
     the complex recurrence into two REAL per-partition scans
        s_j = r*s_{j-1} + e_j   (tensor_tensor_scan, fp32 state),
     with initial s_{-1} = e^{i theta} * h_1023.
  4. Untwist h = e^{+i j theta} (.) s, then output GEMMs:
        y^T = C_re^T.T @ h_re + (-C_im^T).T @ h_im + D^T.T @ x^T

Perf notes (v4):
  - Twists/untwists run full-width [P, 1024]; all elementwise work on DVE
    (GPSIMD compute and DMA-accum combines both measured slower end-to-end).
  - Loads ride ONLY the two HWDGE queues (sync/scalar) in need-time order;
    the gpsimd SWDGE queue measured 3-5x slower and only takes stores.
  - The carry path (wvv = xw|vre|vim) is front-loaded so the first scan is
    not DMA-gated; the W-GEMM runs before the first input GEMM on PE.
  - btiN (host-negated gamma*B_im) removes the negated-wim evac copies from
    the Act engine's critical early window.
"""
import numpy as np

import concourse.bass as bass
import concourse.bacc as bacc
import concourse.mybir as mybir
from concourse.tile import TileContext
from concourse.bass_utils import run_bass_kernel_spmd

BATCH, T, IN, OUT, N = 8, 2048, 512, 512, 512
DS = 1024
P = 128
NB = N // P    # 4 state blocks
IBN = IN // P  # 4 input blocks
OBN = OUT // P # 4 output blocks
HF = 1024      # window length (= DS)
HH = 512       # half length (PSUM bank limit for f32 out)
WCON = 256     # carry W-GEMM contraction (last WCON steps of first half)
WB = WCON // P

f32 = mybir.dt.float32
f16 = mybir.dt.float16
AOP = mybir.AluOpType

_CACHE = {}


def _build_nc():
    if "nc" in _CACHE:
        return _CACHE["nc"]
    nc = bacc.Bacc()
    xT = nc.dram_tensor("xT", [IN, HF], f16, kind="ExternalInput")    # 2nd half, [i, t]
    # wvv rows s: [ xw[s, 0:IN] | vre[s, 0:N] | vim[s, 0:N] ]
    wvv = nc.dram_tensor("wvv", [WCON, IN + 2 * N], f16, kind="ExternalInput")
    # nb-blocked weights: [p, nb*IN] with cols (nb, ib, n-within-block)
    bt01 = nc.dram_tensor("bt01", [P, 2 * IN], f16, kind="ExternalInput")   # btrP0|btiP0
    bt123 = nc.dram_tensor("bt123", [P, 6 * IN], f16, kind="ExternalInput") # btrP1-3|btiP1-3
    btiN = nc.dram_tensor("btiN", [P, 4 * IN], f16, kind="ExternalInput")   # -btiP all nb
    # per-nb trig: rows 0-127 cos, 128-255 sin (so one consolidated DMA each)
    trig = [nc.dram_tensor(f"trig{nb}", [2 * P, HF], f16, kind="ExternalInput")
            for nb in range(NB)]
    eye = nc.dram_tensor("eye", [P, P], f16, kind="ExternalInput")
    # rr columns: 0=r, 1=cos(theta), 2=-sin(theta), 3=sin(theta)
    rr = nc.dram_tensor("rr", [N, 4], f32, kind="ExternalInput")
    ctr = nc.dram_tensor("ctr", [N, OUT], f16, kind="ExternalInput")
    ctin = nc.dram_tensor("ctin", [N, OUT], f16, kind="ExternalInput")
    dtw = nc.dram_tensor("dtw", [IN, OUT], f16, kind="ExternalInput")
    yT = nc.dram_tensor("yT", [OUT, DS], f16, kind="ExternalOutput")

    with TileContext(nc) as tc:
        with (
            tc.tile_pool(name="const", bufs=1) as cp,
            tc.tile_pool(name="work", bufs=1) as wkp,
            tc.tile_pool(name="ps", bufs=2, space="PSUM") as bp,
            tc.tile_pool(name="wps", bufs=2, space="PSUM") as wp,
            tc.tile_pool(name="yps", bufs=4, space="PSUM") as yp,
        ):
            def load_consolidated(dram, eng, tagp, cols=None, col0=0):
                """One rearranged DMA for a [R*P, C] dram table -> row tiles."""
                rows = dram.shape[0]
                nb_ = rows // P
                cols = cols if cols is not None else dram.shape[1]
                big = cp.tile([P, nb_ * cols], dram.dtype, tag=tagp, name=tagp)
                eng.dma_start(
                    big[:].rearrange("p (b c) -> p b c", b=nb_),
                    dram[:, col0:col0 + cols].rearrange("(b p) c -> p b c", p=P))
                return [big[:, i * cols:(i + 1) * cols] for i in range(nb_)]

            trig_t = [None] * NB

            def load_trig(nb, eng):
                big = cp.tile([P, 2 * HF], f16, tag=f"trig{nb}", name=f"trig{nb}")
                eng.dma_start(
                    big[:].rearrange("p (b c) -> p b c", b=2),
                    trig[nb][:, :].rearrange("(b p) c -> p b c", p=P))
                trig_t[nb] = big

            # ---------------- DMA map (need-time order per HWDGE queue) ---
            # scalar (Act HWDGE): carry path + first-block weights first; the
            # queue drains before Act's evac work begins.
            wvv_t = load_consolidated(wvv, nc.scalar, "wvv")  # WB blocks of 1536
            bt01_t = cp.tile([P, 2 * IN], f16, tag="bt01", name="bt01")
            nc.scalar.dma_start(bt01_t[:], bt01[:, :])
            load_trig(0, nc.scalar)
            btiN_t = cp.tile([P, 4 * IN], f16, tag="btiN", name="btiN")
            nc.scalar.dma_start(btiN_t[:], btiN[:, :])
            load_trig(2, nc.scalar)
            ctin_t = load_consolidated(ctin, nc.scalar, "ctin")

            # sync (SP HWDGE): small consts, x halves, remaining weights
            eye_t = cp.tile([P, P], f16, tag="eye", name="eye")
            nc.sync.dma_start(eye_t[:], eye[:, :])
            rr_t = load_consolidated(rr, nc.sync, "rr")     # [P, 4] per nb
            xth0s = load_consolidated(xT, nc.sync, "xth0", cols=HH, col0=0)
            xth1s = load_consolidated(xT, nc.sync, "xth1", cols=HH, col0=HH)
            bt123_t = cp.tile([P, 6 * IN], f16, tag="bt123", name="bt123")
            nc.sync.dma_start(bt123_t[:], bt123[:, :])
            load_trig(1, nc.sync)
            ctr_t = load_consolidated(ctr, nc.sync, "ctr")
            load_trig(3, nc.sync)
            dtw_t = load_consolidated(dtw, nc.sync, "dtw")

            # weight block views
            btrP_t = [bt01_t[:, 0:IN]] + [bt123_t[:, i * IN:(i + 1) * IN]
                                          for i in range(3)]
            btiP_t = [bt01_t[:, IN:2 * IN]] + [bt123_t[:, (3 + i) * IN:(4 + i) * IN]
                                               for i in range(3)]
            btiN_t_ = [btiN_t[:, i * IN:(i + 1) * IN] for i in range(NB)]
            xw_t = [wvv_t[sb][:, 0:IN] for sb in range(WB)]
            vre_t = [wvv_t[sb][:, IN:IN + N] for sb in range(WB)]
            vim_t = [wvv_t[sb][:, IN + N:IN + 2 * N] for sb in range(WB)]

            def xv(ib, h):
                """x^T [P, HH] view for input block ib, half h."""
                return (xth0s if h == 0 else xth1s)[ib]

            def cosv(nb):
                return trig_t[nb][:, 0:HF]

            def sinv(nb):
                return trig_t[nb][:, HF:2 * HF]

            # ---------------- PE building blocks ----------------
            bus = {}

            def in_plane(nb, pi, evac_vec=False):
                wt = (btrP_t, btiP_t)[pi][nb]
                bu = wkp.tile([P, HF], f16, tag=f"bu{pi}", bufs=2,
                              name=f"bu{pi}_{nb}")
                for h in range(2):
                    ps = bp.tile([P, HH], f32, tag="ps", name=f"bups{h}")
                    for ib in range(IBN):
                        nc.tensor.matmul(
                            ps[:], wt[:, ib * P:(ib + 1) * P], xv(ib, h),
                            start=(ib == 0), stop=(ib == IBN - 1))
                    if evac_vec:
                        nc.vector.tensor_scalar_mul(
                            bu[:, h * HH:(h + 1) * HH], ps[:], 1.0)
                    else:
                        nc.scalar.copy(bu[:, h * HH:(h + 1) * HH], ps[:])
                bus[(nb, pi)] = bu

            # W-GEMM (flipped): w[i-block, n] = sum_s xw[s, i] * V[s, n]
            wts = {}

            def w_mm(pi):
                vt = (vre_t, vim_t)[pi]
                for ib in range(IBN):
                    ps = wp.tile([P, IN], f32, tag="wps", name="wps")
                    for sb in range(WB):
                        nc.tensor.matmul(
                            ps[:], xw_t[sb][:, ib * P:(ib + 1) * P], vt[sb][:],
                            start=(sb == 0), stop=(sb == WB - 1))
                    w = wkp.tile([P, IN], f16, tag=f"w{pi}{ib}", name=f"w{pi}{ib}")
                    nc.scalar.copy(w[:], ps[:])
                    wts[(pi, ib)] = w

            inits = {}
            diag_aps = {}

            def diag_mm(nb):
                """a_re/a_im via diag-block MMs + masked STT reduce + rotate."""
                nsl = slice(nb * P, (nb + 1) * P)
                bsl = lambda t, ib: t[:, ib * P:(ib + 1) * P]
                aps = {}
                for name in ("are", "aim"):
                    aps[name] = yp.tile([P, HH], f32, tag="yps", name=name)
                # a_re = sum gbr.wre - gbi.wim  (minus via host-negated btiN)
                # a_im = sum gbr.wim + gbi.wre
                for ib in range(IBN):
                    nc.tensor.matmul(aps["are"][:, 0:P], bsl(btrP_t[nb], ib),
                                     wts[(0, ib)][:, nsl],
                                     start=(ib == 0), stop=False,
                                     skip_group_check=True)
                    nc.tensor.matmul(aps["aim"][:, 0:P], bsl(btrP_t[nb], ib),
                                     wts[(1, ib)][:, nsl],
                                     start=(ib == 0), stop=False,
                                     skip_group_check=True)
                for ib in range(IBN):
                    nc.tensor.matmul(aps["are"][:, 0:P], bsl(btiN_t_[nb], ib),
                                     wts[(1, ib)][:, nsl],
                                     start=False, stop=(ib == IBN - 1),
                                     skip_group_check=True)
                    nc.tensor.matmul(aps["aim"][:, 0:P], bsl(btiP_t[nb], ib),
                                     wts[(0, ib)][:, nsl],
                                     start=False, stop=(ib == IBN - 1),
                                     skip_group_check=True)
                diag_aps[nb] = aps

            def diag_fin(nb):
                aps = diag_aps[nb]
                a_re = cp.tile([P, 1], f32, tag=f"are{nb}", name=f"are{nb}")
                a_im = cp.tile([P, 1], f32, tag=f"aim{nb}", name=f"aim{nb}")
                dre = wkp.tile([P, P], f16, tag="dre", name="dre")
                dim = wkp.tile([P, P], f16, tag="dim", name="dim")
                nc.vector.scalar_tensor_tensor(
                    dre[:], aps["are"][:, 0:P], 1.0, eye_t[:], AOP.bypass,
                    AOP.mult, accum_out=a_re[:])
                nc.vector.scalar_tensor_tensor(
                    dim[:], aps["aim"][:, 0:P], 1.0, eye_t[:], AOP.bypass,
                    AOP.mult, accum_out=a_im[:])
                # init = e^{i theta} * a
                rt = rr_t[nb]
                i_re = cp.tile([P, 1], f32, tag=f"ire{nb}", name=f"ire{nb}")
                i_im = cp.tile([P, 1], f32, tag=f"iim{nb}", name=f"iim{nb}")
                u_re = cp.tile([P, 1], f32, tag=f"ure{nb}", name=f"ure{nb}")
                u_im = cp.tile([P, 1], f32, tag=f"uim{nb}", name=f"uim{nb}")
                nc.scalar.mul(u_re[:], a_re[:], rt[:, 1:2])
                nc.vector.scalar_tensor_tensor(
                    i_re[:], a_im[:], rt[:, 2:3], u_re[:], AOP.mult, AOP.add)
                nc.scalar.mul(u_im[:], a_im[:], rt[:, 1:2])
                nc.vector.scalar_tensor_tensor(
                    i_im[:], a_re[:], rt[:, 3:4], u_im[:], AOP.mult, AOP.add)
                inits[nb] = (i_re, i_im)

            # ---------------- DVE building blocks (full width) -----------
            es = {}

            def twist(nb, h=None):
                ct, st = cosv(nb), sinv(nb)
                br, bi = bus[(nb, 0)], bus[(nb, 1)]
                if h is None or h == 0:
                    e_re = wkp.tile([P, HF], f16, tag="er", bufs=4, name=f"er{nb}")
                    e_im = wkp.tile([P, HF], f16, tag="ei", bufs=4, name=f"ei{nb}")
                    es[nb] = (e_re, e_im)
                e_re, e_im = es[nb]
                sl = slice(0, HF) if h is None else slice(h * HH, (h + 1) * HH)
                W = HF if h is None else HH
                sfx = "f" if h is None else "h"
                p1 = wkp.tile([P, W], f16, tag=f"p1{sfx}", bufs=2, name="p1")
                p2 = wkp.tile([P, W], f16, tag=f"p2{sfx}", bufs=2, name="p2")
                nc.vector.tensor_tensor(p1[:], st[:, sl], bi[:, sl], AOP.mult)
                nc.vector.tensor_tensor(p2[:], ct[:, sl], br[:, sl], AOP.mult)
                nc.vector.tensor_tensor(e_re[:, sl], p2[:], p1[:], AOP.add)
                nc.vector.tensor_tensor(p1[:], st[:, sl], br[:, sl], AOP.mult)
                nc.vector.tensor_tensor(p2[:], ct[:, sl], bi[:, sl], AOP.mult)
                nc.vector.tensor_tensor(e_im[:, sl], p2[:], p1[:], AOP.subtract)

            ss = {}

            def scan(nb):
                i_re, i_im = inits[nb]
                e_re, e_im = es[nb]
                s_re = wkp.tile([P, HF], f16, tag="sr", bufs=4, name=f"sr{nb}")
                s_im = wkp.tile([P, HF], f16, tag="si", bufs=4, name=f"si{nb}")
                ss[nb] = (s_re, s_im)
                rbb = rr_t[nb][:, 0:1].broadcast_to((P, HF))
                nc.vector.tensor_tensor_scan(
                    s_re[:], rbb, e_re[:], i_re[:, 0:1], AOP.mult, AOP.add)
                nc.vector.tensor_tensor_scan(
                    s_im[:], rbb, e_im[:], i_im[:, 0:1], AOP.mult, AOP.add)

            hhs = {}

            def untwist(nb, h=None):
                s_re, s_im = ss[nb]
                ct, st = cosv(nb), sinv(nb)
                if h is None or h == 0:
                    hhr = wkp.tile([P, HF], f16, tag="hhr", bufs=4, name=f"hhr{nb}")
                    hhi = wkp.tile([P, HF], f16, tag="hhi", bufs=4, name=f"hhi{nb}")
                    hhs[nb] = (hhr, hhi)
                hhr, hhi = hhs[nb]
                sl = slice(0, HF) if h is None else slice(h * HH, (h + 1) * HH)
                W = HF if h is None else HH
                sfx = "f" if h is None else "h"
                q1 = wkp.tile([P, W], f16, tag=f"q1{sfx}", bufs=2, name="q1")
                q2 = wkp.tile([P, W], f16, tag=f"q2{sfx}", bufs=2, name="q2")
                nc.vector.tensor_tensor(q1[:], ct[:, sl], s_re[:, sl], AOP.mult)
                nc.vector.tensor_tensor(q2[:], st[:, sl], s_im[:, sl], AOP.mult)
                nc.vector.tensor_tensor(hhr[:, sl], q1[:], q2[:], AOP.subtract)
                nc.vector.tensor_tensor(q1[:], ct[:, sl], s_im[:, sl], AOP.mult)
                nc.vector.tensor_tensor(q2[:], st[:, sl], s_re[:, sl], AOP.mult)
                nc.vector.tensor_tensor(hhi[:, sl], q1[:], q2[:], AOP.add)

            # ---------------- output groups ----------------
            groups = {}

            def d_gemms():
                for h in range(2):
                    for ob in range(OBN):
                        gi = h * OBN + ob
                        pool, tag = ((yp, "yps") if gi < 4 else
                                     (bp, "ps") if gi < 6 else (wp, "wps"))
                        groups[(h, ob)] = pool.tile([P, HH], f32, tag=tag, name="yps")
                for ob in range(OBN):
                    osl = slice(ob * P, (ob + 1) * P)
                    for ib in range(IBN):
                        for h in range(2):
                            nc.tensor.matmul(
                                groups[(h, ob)][:], dtw_t[ib][:, osl], xv(ib, h),
                                start=(ib == 0), stop=False)

            store_eng = [nc.sync, nc.gpsimd, nc.scalar]

            def store_group(h, ob):
                osl = slice(ob * P, (ob + 1) * P)
                hsl = slice(h * HH, (h + 1) * HH)
                ysb = wkp.tile([P, HH], f16, tag="ysb", bufs=4, name="ysb")
                nc.scalar.copy(ysb[:], groups[(h, ob)][:])
                eng = store_eng[(h * OBN + ob) % 3]
                eng.dma_start(yT[osl, hsl], ysb[:])

            def c_mms(nb, hs=(0, 1)):
                hhr, hhi = hhs[nb]
                last = nb == NB - 1
                for h in hs:
                    for ob in range(OBN):
                        osl = slice(ob * P, (ob + 1) * P)
                        for wi, (wt, m) in enumerate(
                                ((ctr_t[nb][:, osl], hhr), (ctin_t[nb][:, osl], hhi))):
                            nc.tensor.matmul(
                                groups[(h, ob)][:], wt, m[:, h * HH:(h + 1) * HH],
                                start=False, stop=(last and wi == 1))
                        if last:
                            store_group(h, ob)

            # ---------------- emission ----------------
            w_mm(0)
            in_plane(0, 0)
            in_plane(0, 1)
            twist(0, 0)
            w_mm(1)
            twist(0, 1)
            in_plane(1, 0)
            in_plane(1, 1)
            diag_mm(0)
            diag_mm(1)
            diag_fin(0)
            scan(0)
            diag_mm(2)
            diag_mm(3)
            in_plane(2, 0)
            in_plane(2, 1)
            twist(1)
            diag_fin(1)
            untwist(0)
            in_plane(3, 0)
            in_plane(3, 1)
            scan(1)
            diag_fin(2)
            diag_fin(3)
            d_gemms()
            c_mms(0)
            twist(2)
            untwist(1)
            c_mms(1)
            scan(2)
            twist(3)
            untwist(2)
            c_mms(2)
            scan(3)
            untwist(3, 0)
            c_mms(3, hs=(0,))
            untwist(3, 1)
            c_mms(3, hs=(1,))

    nc.compile()
    nc.finalize()
    _CACHE["nc"] = nc
    return nc


def _host_prep(x, nu_log, theta_log, gamma_log, B_re, B_im, C_re, C_im, D):
    f64 = np.float64
    nu = np.asarray(nu_log, f64)
    th = np.asarray(theta_log, f64)
    gl = np.asarray(gamma_log, f64)
    r = np.exp(-np.exp(nu))
    theta = np.exp(th)
    gamma = np.exp(gl)

    gbr = gamma[:, None] * np.asarray(B_re, f64)   # [n, i]
    gbi = gamma[:, None] * np.asarray(B_im, f64)

    def blocked(w_ni):
        """[n, i] -> [p, (nb, ib, nl)] with p = i within block."""
        wt = np.ascontiguousarray(w_ni.T)  # [i, n]
        b = wt.reshape(IBN, P, NB, P)      # (ib, p, nb, nl)
        return np.ascontiguousarray(b.transpose(1, 2, 0, 3).reshape(P, NB * IN)
                                    ).astype(np.float16)

    btrP = blocked(gbr)
    btiP = blocked(gbi)
    shared = {
        "bt01": np.ascontiguousarray(
            np.concatenate([btrP[:, 0:IN], btiP[:, 0:IN]], axis=1)),
        "bt123": np.ascontiguousarray(
            np.concatenate([btrP[:, IN:], btiP[:, IN:]], axis=1)),
        "btiN": np.ascontiguousarray(blocked(-gbi)),
        "ctr": np.ascontiguousarray(np.asarray(C_re, f64).T).astype(np.float16),
        "ctin": np.ascontiguousarray((-np.asarray(C_im, f64)).T).astype(np.float16),
        "dtw": np.ascontiguousarray(np.asarray(D, f64).T).astype(np.float16),
        "eye": np.eye(P, dtype=np.float16),
    }
    j = np.arange(HF, dtype=f64)
    ang = theta[:, None] * j[None, :]
    cosj = np.cos(ang).astype(np.float16)   # [N, HF]
    sinj = np.sin(ang).astype(np.float16)
    for nb in range(NB):
        nsl = slice(nb * P, (nb + 1) * P)
        shared[f"trig{nb}"] = np.ascontiguousarray(
            np.concatenate([cosj[nsl], sinj[nsl]], axis=0))
    # V = lam^{WCON-1-s} over the LAST WCON steps of the first half, [s, n]
    e = (WCON - 1) - np.arange(WCON, dtype=f64)
    mag = np.exp(np.log(r)[:, None] * e[None, :])
    angv = theta[:, None] * e[None, :]
    vre = np.ascontiguousarray((mag * np.cos(angv)).T).astype(np.float16)
    vim = np.ascontiguousarray((mag * np.sin(angv)).T).astype(np.float16)
    shared["rr"] = np.ascontiguousarray(np.stack(
        [r, np.cos(theta), -np.sin(theta), np.sin(theta)],
        axis=1).astype(np.float32))

    x = np.asarray(x, np.float32)
    in_maps = []
    for b in range(BATCH):
        m = dict(shared)
        m["xT"] = np.ascontiguousarray(x[b, HF:].T).astype(np.float16)
        xw = np.ascontiguousarray(x[b, HF - WCON:HF]).astype(np.float16)
        m["wvv"] = np.ascontiguousarray(
            np.concatenate([xw, vre, vim], axis=1))
        in_maps.append(m)
    return in_maps


def _run(in_maps, trace=False):
    nc = _build_nc()
    return run_bass_kernel_spmd(nc, in_maps, core_ids=list(range(BATCH)), trace=trace)


def kernel(**inputs):
    in_maps = _host_prep(**inputs)
    res = _run(in_maps, trace=False)
    y = np.stack([np.ascontiguousarray(res.results[b]["yT"].T) for b in range(BATCH)])
    return y.astype(np.float32)


def kernel_traced(**inputs):
    """Like kernel() but returns (y, exec_time_ns). Used by test.py."""
    in_maps = _host_prep(**inputs)
    res = _run(in_maps, trace=True)
    y = np.stack([np.ascontiguousarray(res.results[b]["yT"].T) for b in range(BATCH)])
    return y.astype(np.float32), res.exec_time_ns
